# revision 4
# baseline (speedup 1.0000x reference)
"""Trainium2 Bass kernel for nn_Attention_45303315038988 (v2).

  q = p @ Wh.T (+bh) ; k = r @ Wl.T + bl ; v = p @ Wg.T + bg     [N, D]
  scores = q @ k.T ; attn = softmax(scores, axis=0) ; out = p + attn @ v

Sequence-parallel over the query axis on 8 cores; all matmuls fp16 with
fp32 PSUM; softmax stats fp32; bh dropped (cancels in softmax over axis 0).

Final (v2 + deeper output-store pool) over the rev-3 baseline; measured
~565-700 ns/iter (machine-load dependent) vs ~790-900 for the rev-3
baseline in interleaved same-process A/B. Changes:
  - k^T is computed column-chunk-major and AllGathered in TWO chunks of
    [D, NL/2], so phase C starts after the first half-AG instead of the
    full 16 MB gather. Phase C iterates chunk-major with remapped group
    index g = ch*32 + c*4 + jl, keeping the stats halves contiguous.
  - Phase-E V-tile loads all ride the sync queue (they were FIFO-stuck
    behind the E-rescale burst on the scalar queue: 23 us PE stall).
  - E-rescale moved from ACT to DVE (tensor_scalar_mul with per-partition
    scalar), freeing the ACT queue for the exp stream.
  - Residual p prefetched on the gpsimd queue during phase E pass 1;
    output stores alternate scalar/sync queues: shorter output tail.
"""
import numpy as np

P = 128
D = 1024
N = 8192
NCORES = 8
NL = N // NCORES
DB = D // P
JBL = NL // P
NG = N // P
IB = NL // P
FH = 512
NCH = 2              # kT AllGather chunks
JC = JBL // NCH      # key blocks per core per chunk (4)
CW = JC * P          # chunk width in keys (512)


def build_nc(k_iters: int = 1, no_cc: bool = False, phases: str = "full",
             opts: dict | None = None, spin_us: int = 0):
    opts = opts or {}
    import concourse.mybir as mybir
    import concourse.tile as tile
    from concourse import bacc

    f16 = mybir.dt.float16
    f32 = mybir.dt.float32
    AF = mybir.ActivationFunctionType
    AX = mybir.AxisListType
    ALU = mybir.AluOpType
    RG = [list(range(NCORES))]

    nc = bacc.Bacc("TRN2", target_bir_lowering=False, debug=False,
                   num_devices=1 if no_cc else NCORES)

    def collective(kind, op, ins, outs):
        if no_cc:
            src_ap, dst_ap = ins[0], outs[0]
            nc.sync.dma_start(out=dst_ap[0] if kind == "AllGather" else dst_ap[:],
                              in_=src_ap[:])
        else:
            nc.gpsimd.collective_compute(kind, op, replica_groups=RG,
                                         ins=[ins[0].opt()], outs=[outs[0].opt()])

    pT_h = nc.dram_tensor("pT", [D, NL], f16, kind="ExternalInput")
    rT_h = nc.dram_tensor("rT", [D, NL], f16, kind="ExternalInput")
    pres_h = nc.dram_tensor("pres", [NL, D], f32, kind="ExternalInput")
    WhT_h = nc.dram_tensor("WhT", [D, D], f16, kind="ExternalInput")
    WlT_h = nc.dram_tensor("WlT", [D, D], f16, kind="ExternalInput")
    WgT_h = nc.dram_tensor("WgT", [D, D], f16, kind="ExternalInput")
    bl_h = nc.dram_tensor("bl_r", [P, DB], f32, kind="ExternalInput")
    bg_h = nc.dram_tensor("bg16", [1, D], f16, kind="ExternalInput")
    ones_h = nc.dram_tensor("ones16", [1, P], f16, kind="ExternalInput")
    out_h = nc.dram_tensor("out", [NL, D], f32, kind="ExternalOutput")

    with tile.TileContext(nc) as tc:
        with tc.tile_pool(name="dram", bufs=1, space="DRAM") as dpool:
            for it in range(k_iters):
                cc_kt_in = [dpool.tile([D, CW], f16, name=f"cc_kt_in{it}_{ch}")
                            for ch in range(NCH)]
                cc_kt_out = [dpool.tile([NCORES, D, CW], f16,
                                        addr_space="Shared",
                                        name=f"cc_kt_out{it}_{ch}")
                             for ch in range(NCH)]
                cc_v_in = dpool.tile([NL, D], f16, name=f"cc_v_in{it}")
                cc_v_out = dpool.tile([NCORES, NL, D], f16,
                                      addr_space="Shared", name=f"cc_v_out{it}")
                cc_st_in = [dpool.tile([P, NG], f32, name=f"cc_st_in{it}_{h}")
                            for h in range(2)]
                cc_st_out = [dpool.tile([NCORES, P, NG], f32, addr_space="Shared",
                                        name=f"cc_st_out{it}_{h}")
                             for h in range(2)]
                cc_wm_in = dpool.tile([P, 8], f32, name=f"cc_wm_in{it}")
                cc_wm_out = dpool.tile([NCORES, P, 8], f32, addr_space="Shared",
                                       name=f"cc_wm_out{it}")

                with tc.tile_pool(name="lp", bufs=1) as lp:
                    qT = lp.tile([P, DB, NL], f16)
                    stats = lp.tile([P, 2, NG], f32)   # [:,0,:]=-max, [:,1,:]=sum
                    f_sc = lp.tile([P, NG], f32)
                    bl_sb = lp.tile([P, DB], f32)
                    bg_sb = lp.tile([1, D], f16)
                    ones_sb = lp.tile([1, P], f16)
                    nc.sync.dma_start(out=bl_sb, in_=bl_h.ap())
                    nc.sync.dma_start(out=bg_sb, in_=bg_h.ap())
                    nc.sync.dma_start(out=ones_sb, in_=ones_h.ap())
                    if it == 0:
                        # warm-up collective: absorbs the cold first-collective
                        # latency so AG(k^T ch0) runs warm
                        nc.gpsimd.dma_start(out=cc_wm_in, in_=bl_h.ap())
                        collective("AllGather", ALU.bypass,
                                   [cc_wm_in], [cc_wm_out])

                    # ---------------- phase A: projections ----------------
                    kt_pre = lp.tile([P, DB, JC, P], f16, name="kt_pre")
                    with (
                        tc.tile_pool(name="pw", bufs=1) as pw,
                        tc.tile_pool(name="pst", bufs=4) as pst,
                        tc.tile_pool(name="psA", bufs=3, space="PSUM") as psA,
                        tc.tile_pool(name="psA2", bufs=2, space="PSUM") as psA2,
                    ):
                        WhT_sb = pw.tile([P, DB, D], f16)
                        WlT_sb = pw.tile([P, DB, D], f16)
                        WgT_sb = pw.tile([P, DB, D], f16)
                        pT_sb = pw.tile([P, DB, NL], f16)
                        rT_sb = pw.tile([P, DB, NL], f16)
                        for db in range(DB):
                            for t_sb, t_h in ((WlT_sb, WlT_h), (rT_sb, rT_h)):
                                nc.sync.dma_start(
                                    out=t_sb[:, db, :],
                                    in_=t_h.ap()[db * P:(db + 1) * P, :])
                        for t_sb, t_h in ((WgT_sb, WgT_h), (pT_sb, pT_h),
                                          (WhT_sb, WhT_h)):
                            for db in range(DB):
                                nc.sync.dma_start(
                                    out=t_sb[:, db, :],
                                    in_=t_h.ap()[db * P:(db + 1) * P, :])

                        # k^T shard chunk-major: chunk ch covers keys
                        # [ch*CW, (ch+1)*CW); AG each chunk as soon as done
                        for ch in range(NCH):
                            for dob in range(DB):
                                ps_t = psA.tile([P, CW], f32, tag="psA")
                                for db in range(DB):
                                    nc.tensor.matmul(
                                        ps_t,
                                        lhsT=WlT_sb[:, db, dob * P:(dob + 1) * P],
                                        rhs=rT_sb[:, db, ch * CW:(ch + 1) * CW],
                                        start=(db == 0), stop=(db == DB - 1))
                                st = pst.tile([P, CW], f16, tag="st")
                                nc.scalar.activation(
                                    out=st, in_=ps_t, func=AF.Identity,
                                    bias=bl_sb[:, dob:dob + 1], scale=1.0)
                                nc.scalar.dma_start(
                                    out=cc_kt_in[ch][dob * P:(dob + 1) * P, :],
                                    in_=st)
                            collective("AllGather", ALU.bypass,
                                       [cc_kt_in[ch]], [cc_kt_out[ch]])
                            if ch == 0:
                                # prefetch first K^T block (ch0, core0) on sync
                                for db in range(DB):
                                    nc.sync.dma_start(
                                        out=kt_pre[:, db, :, :].rearrange(
                                            "p a b -> p (a b)"),
                                        in_=cc_kt_out[0][0, db * P:(db + 1) * P, :])

                        # v shard = p^T.T @ Wg^T -> [j_l, dv], + bg via ones-row
                        for jb in range(JBL):
                            ps_t = psA2.tile([P, NL], f32, tag="psAv")
                            for db in range(DB):
                                for dvh in range(2):
                                    nc.tensor.matmul(
                                        ps_t[:, dvh * FH:(dvh + 1) * FH],
                                        lhsT=pT_sb[:, db, jb * P:(jb + 1) * P],
                                        rhs=WgT_sb[:, db, dvh * FH:(dvh + 1) * FH],
                                        start=(db == 0), stop=False)
                            for dvh in range(2):
                                nc.tensor.matmul(
                                    ps_t[:, dvh * FH:(dvh + 1) * FH],
                                    lhsT=ones_sb[:, :],
                                    rhs=bg_sb[:, dvh * FH:(dvh + 1) * FH],
                                    start=False, stop=True)
                            st = pst.tile([P, NL], f16, tag="stv")
                            nc.scalar.activation(out=st, in_=ps_t, func=AF.Copy)
                            nc.scalar.dma_start(
                                out=cc_v_in[jb * P:(jb + 1) * P, :], in_=st)
                        collective("AllGather", ALU.bypass, [cc_v_in], [cc_v_out])

                        # q^T = Wh^T.T @ p^T -> [do, i]; stays in SBUF
                        for dob in range(DB):
                            ps_t = psA2.tile([P, NL], f32, tag="psAv")
                            for db in range(DB):
                                for ih in range(2):
                                    nc.tensor.matmul(
                                        ps_t[:, ih * FH:(ih + 1) * FH],
                                        lhsT=WhT_sb[:, db, dob * P:(dob + 1) * P],
                                        rhs=pT_sb[:, db, ih * FH:(ih + 1) * FH],
                                        start=(db == 0), stop=(db == DB - 1))
                            nc.scalar.activation(out=qT[:, dob, :], in_=ps_t,
                                                 func=AF.Copy)

                    # -------- phase C: scores^T + local stats --------
                    # group order: g = ch*32 + c_idx*JC + jl  (chunk-major)
                    ep_cm = tc.tile_pool(name="ep", bufs=1)
                    ep = ep_cm.__enter__()
                    E = ep.tile([P, NG, NL], f16)
                    with (
                        tc.tile_pool(name="ktp", bufs=3) as ktp,
                        tc.tile_pool(name="psC", bufs=3, space="PSUM") as psC,
                    ):
                        for ch in range(NCH):
                            for c_idx in range(NCORES):
                                if ch == 0 and c_idx == 0:
                                    kt_c = kt_pre
                                else:
                                    kt_c = ktp.tile([P, DB, JC, P], f16, tag="kt")
                                    for db in range(DB):
                                        nc.sync.dma_start(
                                            out=kt_c[:, db, :, :].rearrange(
                                                "p a b -> p (a b)"),
                                            in_=cc_kt_out[ch][
                                                c_idx, db * P:(db + 1) * P, :])
                                for jl in range(JC):
                                    g = ch * (NG // 2) + c_idx * JC + jl
                                    ps_t = psC.tile([P, NL], f32, tag="sc")
                                    for db in range(DB):
                                        for ih in range(2):
                                            nc.tensor.matmul(
                                                ps_t[:, ih * FH:(ih + 1) * FH],
                                                lhsT=kt_c[:, db, jl, :],
                                                rhs=qT[:, db, ih * FH:(ih + 1) * FH],
                                                start=(db == 0),
                                                stop=(db == DB - 1))
                                    nc.vector.tensor_reduce(
                                        out=stats[:, 0, g:g + 1], in_=ps_t,
                                        op=ALU.max, axis=AX.X, negate=True)
                                    nc.scalar.activation(
                                        out=E[:, g, :], in_=ps_t, func=AF.Exp,
                                        bias=stats[:, 0, g:g + 1], scale=1.0,
                                        accum_out=stats[:, 1, g:g + 1])

                    # prefetch first phase-E V tiles (sync queue)
                    vtp_cm = tc.tile_pool(name="vtp", bufs=8)
                    vtp = vtp_cm.__enter__()

                    def v_src(g):
                        c_idx = (g % (NG // 2)) // JC
                        jlb = (g // (NG // 2)) * JC + (g % JC)
                        return cc_v_out[c_idx, jlb * P:(jlb + 1) * P, :]

                    vt_pre = []
                    for g in range(6):
                        vt = vtp.tile([P, D], f16, tag="vt", name=f"vtpre{g}")
                        nc.sync.dma_start(out=vt, in_=v_src(g))
                        vt_pre.append(vt)

                    # residual p prefetch (per i-half) on gpsimd: needed only
                    # at pass ends
                    prp_cm = tc.tile_pool(name="prp", bufs=1)
                    prp = prp_cm.__enter__()

                    # stats AllGather + combine in two halves; the first
                    # half's AG/combine/E-scale hide under phase C's tail
                    NH = NG // 2
                    Mneg = lp.tile([P, NG], f32)
                    Ssum = lp.tile([P, NG], f32)
                    tmp = lp.tile([P, NG], f32)
                    diff = lp.tile([P, NG], f32)
                    alpha = lp.tile([P, NG], f32)
                    rec = lp.tile([P, NG], f32)
                    gath = [lp.tile([P, NCORES, 2, NH], f32, name=f"gath{h}")
                            for h in range(2)]
                    for h in range(2):
                        hs = slice(h * NH, (h + 1) * NH)
                        nc.gpsimd.dma_start(out=cc_st_in[h][:, 0:NH],
                                            in_=stats[:, 0, hs])
                        nc.gpsimd.dma_start(out=cc_st_in[h][:, NH:NG],
                                            in_=stats[:, 1, hs])
                        collective("AllGather", ALU.bypass,
                                   [cc_st_in[h]], [cc_st_out[h]])
                        nc.gpsimd.dma_start(
                            out=gath[h].rearrange("p c a b -> p (c a b)"),
                            in_=cc_st_out[h].rearrange("c p x -> p c x"))
                        g_h = gath[h]
                        nc.vector.tensor_copy(out=Mneg[:, hs],
                                              in_=g_h[:, 0, 0, :])
                        for c in range(1, NCORES):
                            nc.vector.tensor_tensor(out=Mneg[:, hs],
                                                    in0=Mneg[:, hs],
                                                    in1=g_h[:, c, 0, :],
                                                    op=ALU.min)
                        for c in range(NCORES):
                            nc.vector.tensor_sub(out=tmp[:, hs],
                                                 in0=Mneg[:, hs],
                                                 in1=g_h[:, c, 0, :])
                            nc.scalar.activation(out=tmp[:, hs],
                                                 in_=tmp[:, hs], func=AF.Exp)
                            nc.vector.tensor_mul(out=tmp[:, hs],
                                                 in0=tmp[:, hs],
                                                 in1=g_h[:, c, 1, :])
                            if c == 0:
                                nc.vector.tensor_copy(out=Ssum[:, hs],
                                                      in_=tmp[:, hs])
                            else:
                                nc.vector.tensor_add(out=Ssum[:, hs],
                                                     in0=Ssum[:, hs],
                                                     in1=tmp[:, hs])
                        # f = exp(Mneg - mneg_local) / Ssum, fold into E (DVE)
                        nc.vector.tensor_sub(out=diff[:, hs], in0=Mneg[:, hs],
                                             in1=stats[:, 0, hs])
                        nc.scalar.activation(out=alpha[:, hs], in_=diff[:, hs],
                                             func=AF.Exp)
                        nc.vector.reciprocal(out=rec[:, hs], in_=Ssum[:, hs])
                        nc.vector.tensor_mul(out=f_sc[:, hs], in0=alpha[:, hs],
                                             in1=rec[:, hs])
                        for g in range(h * NH, (h + 1) * NH):
                            nc.vector.tensor_scalar_mul(
                                E[:, g, :], E[:, g, :], f_sc[:, g:g + 1])

                    # -------- phase E: out = E^T.T @ V + p (two i-half passes)
                    with (
                        tc.tile_pool(name="osp", bufs=3) as osp,
                        tc.tile_pool(name="psE", bufs=1, space="PSUM") as psE,
                    ):
                        for ihalf in range(2):
                            po = [psE.tile([P, D], f32, tag=f"po{q_}",
                                           name=f"po{q_}")
                                  for q_ in range(IB // 2)]
                            pr_sb = prp.tile([P, IB // 2, D], f32, tag="pr")
                            for q_ in range(IB // 2):
                                ib = ihalf * (IB // 2) + q_
                                nc.gpsimd.dma_start(
                                    out=pr_sb[:, q_, :],
                                    in_=pres_h.ap()[ib * P:(ib + 1) * P, :])
                            for g in range(NG):
                                if ihalf == 0 and g < 6:
                                    vt = vt_pre[g]
                                else:
                                    vt = vtp.tile([P, D], f16, tag="vt")
                                    nc.sync.dma_start(out=vt, in_=v_src(g))
                                for q_ in range(IB // 2):
                                    ib = ihalf * (IB // 2) + q_
                                    for dvh in range(2):
                                        nc.tensor.matmul(
                                            po[q_][:, dvh * FH:(dvh + 1) * FH],
                                            lhsT=E[:, g, ib * P:(ib + 1) * P],
                                            rhs=vt[:, dvh * FH:(dvh + 1) * FH],
                                            start=(g == 0), stop=(g == NG - 1))
                            for q_ in range(IB // 2):
                                ib = ihalf * (IB // 2) + q_
                                ot = osp.tile([P, D], f32, tag="ot")
                                nc.vector.tensor_add(out=ot, in0=po[q_],
                                                     in1=pr_sb[:, q_, :])
                                eng = nc.scalar if q_ % 2 == 0 else nc.sync
                                eng.dma_start(
                                    out=out_h.ap()[ib * P:(ib + 1) * P, :],
                                    in_=ot)
                    prp_cm.__exit__(None, None, None)
                    vtp_cm.__exit__(None, None, None)
                    ep_cm.__exit__(None, None, None)
            if spin_us:
                with tc.tile_critical():
                    for _ in range(spin_us):
                        nc.vector.nop(cycle_cnt=960)
    nc.compile()
    return nc


def prepare_in_maps(p, r, Wh, bh, Wl, bl, Wg, bg):
    f16 = np.float16
    f32 = np.float32
    WhT = np.ascontiguousarray(Wh.T).astype(f16)
    WlT = np.ascontiguousarray(Wl.T).astype(f16)
    WgT = np.ascontiguousarray(Wg.T).astype(f16)
    bl_r = np.ascontiguousarray(bl.astype(f32).reshape(DB, P).T)
    bg16 = bg.astype(f16).reshape(1, D)
    in_maps = []
    for c in range(NCORES):
        sl = slice(c * NL, (c + 1) * NL)
        in_maps.append({
            "pT": np.ascontiguousarray(p[sl].T).astype(f16),
            "rT": np.ascontiguousarray(r[sl].T).astype(f16),
            "pres": np.ascontiguousarray(p[sl]).astype(f32),
            "WhT": WhT, "WlT": WlT, "WgT": WgT,
            "bl_r": bl_r, "bg16": bg16, "ones16": np.ones((1, P), f16),
        })
    return in_maps


_NC_CACHE = {}


def kernel(p, r, Wh, bh, Wl, bl, Wg, bg):
    from concourse.bass_utils import run_bass_kernel_spmd

    p = np.asarray(p); r = np.asarray(r)
    in_maps = prepare_in_maps(p, r, np.asarray(Wh), np.asarray(bh),
                              np.asarray(Wl), np.asarray(bl),
                              np.asarray(Wg), np.asarray(bg))
    if 1 not in _NC_CACHE:
        _NC_CACHE[1] = build_nc(1)
    res = run_bass_kernel_spmd(_NC_CACHE[1], in_maps, list(range(NCORES)))
    out = np.concatenate([res.results[c]["out"] for c in range(NCORES)], axis=0)
    return out.astype(np.float32)


# revision 5
# speedup vs baseline: 1.1468x; 1.1468x over previous
"""Trainium2 Bass kernel for nn_Attention_45303315038988 (v2).

  q = p @ Wh.T (+bh) ; k = r @ Wl.T + bl ; v = p @ Wg.T + bg     [N, D]
  scores = q @ k.T ; attn = softmax(scores, axis=0) ; out = p + attn @ v

Sequence-parallel over the query axis on 8 cores; all matmuls fp16 with
fp32 PSUM; softmax stats fp32; bh dropped (cancels in softmax over axis 0).

v2 over the rev-3 baseline:
  - k^T is computed column-chunk-major and AllGathered in TWO chunks of
    [D, NL/2], so phase C starts after the first half-AG instead of the
    full 16 MB gather. Phase C iterates chunk-major with remapped group
    index g = ch*32 + c*4 + jl, keeping the stats halves contiguous.
  - Phase-E V-tile loads all ride the sync queue (they were FIFO-stuck
    behind the E-rescale burst on the scalar queue: 23 us PE stall).
  - E-rescale moved from ACT to DVE (tensor_scalar_mul with per-partition
    scalar), freeing the ACT queue for the exp stream.
  - Residual p prefetched on the gpsimd queue during phase E pass 1;
    output stores alternate scalar/sync queues: shorter output tail.
"""
import numpy as np

P = 128
D = 1024
N = 8192
NCORES = 8
NL = N // NCORES
DB = D // P
JBL = NL // P
NG = N // P
IB = NL // P
FH = 512
NCH = 2              # kT AllGather chunks
JC = JBL // NCH      # key blocks per core per chunk (4)
CW = JC * P          # chunk width in keys (512)


def build_nc(k_iters: int = 1, no_cc: bool = False, phases: str = "full",
             opts: dict | None = None, spin_us: int = 0):
    opts = opts or {}
    import concourse.mybir as mybir
    import concourse.tile as tile
    from concourse import bacc

    f16 = mybir.dt.float16
    f32 = mybir.dt.float32
    AF = mybir.ActivationFunctionType
    AX = mybir.AxisListType
    ALU = mybir.AluOpType
    RG = [list(range(NCORES))]

    nc = bacc.Bacc("TRN2", target_bir_lowering=False, debug=False,
                   num_devices=1 if no_cc else NCORES)

    def collective(kind, op, ins, outs):
        if no_cc:
            src_ap, dst_ap = ins[0], outs[0]
            nc.sync.dma_start(out=dst_ap[0] if kind == "AllGather" else dst_ap[:],
                              in_=src_ap[:])
        else:
            nc.gpsimd.collective_compute(kind, op, replica_groups=RG,
                                         ins=[ins[0].opt()], outs=[outs[0].opt()])

    pT_h = nc.dram_tensor("pT", [D, NL], f16, kind="ExternalInput")
    rT_h = nc.dram_tensor("rT", [D, NL], f16, kind="ExternalInput")
    pres_h = nc.dram_tensor("pres", [NL, D], f32, kind="ExternalInput")
    WhT_h = nc.dram_tensor("WhT", [D, D], f16, kind="ExternalInput")
    WlT_h = nc.dram_tensor("WlT", [D, D], f16, kind="ExternalInput")
    WgT_h = nc.dram_tensor("WgT", [D, D], f16, kind="ExternalInput")
    bl_h = nc.dram_tensor("bl_r", [P, DB], f32, kind="ExternalInput")
    bg_h = nc.dram_tensor("bg16", [1, D], f16, kind="ExternalInput")
    ones_h = nc.dram_tensor("ones16", [1, P], f16, kind="ExternalInput")
    out_h = nc.dram_tensor("out", [NL, D], f32, kind="ExternalOutput")

    with tile.TileContext(nc) as tc:
        with tc.tile_pool(name="dram", bufs=1, space="DRAM") as dpool:
            for it in range(k_iters):
                cc_kt_in = [dpool.tile([D, CW], f16, name=f"cc_kt_in{it}_{ch}")
                            for ch in range(NCH)]
                cc_kt_out = [dpool.tile([NCORES, D, CW], f16,
                                        addr_space="Shared",
                                        name=f"cc_kt_out{it}_{ch}")
                             for ch in range(NCH)]
                cc_v_in = dpool.tile([NL, D], f16, name=f"cc_v_in{it}")
                cc_v_out = dpool.tile([NCORES, NL, D], f16,
                                      addr_space="Shared", name=f"cc_v_out{it}")
                cc_st_in = [dpool.tile([P, NG], f32, name=f"cc_st_in{it}_{h}")
                            for h in range(2)]
                cc_st_out = [dpool.tile([NCORES, P, NG], f32, addr_space="Shared",
                                        name=f"cc_st_out{it}_{h}")
                             for h in range(2)]
                cc_wm_in = dpool.tile([P, 8], f32, name=f"cc_wm_in{it}")
                cc_wm_out = dpool.tile([NCORES, P, 8], f32, addr_space="Shared",
                                       name=f"cc_wm_out{it}")

                with tc.tile_pool(name="lp", bufs=1) as lp:
                    qT = lp.tile([P, DB, NL], f16)
                    stats = lp.tile([P, 2, NG], f32)   # [:,0,:]=-max, [:,1,:]=sum
                    f_sc = lp.tile([P, NG], f32)
                    bl_sb = lp.tile([P, DB], f32)
                    bg_sb = lp.tile([1, D], f16)
                    ones_sb = lp.tile([1, P], f16)
                    nc.sync.dma_start(out=bl_sb, in_=bl_h.ap())
                    nc.sync.dma_start(out=bg_sb, in_=bg_h.ap())
                    nc.sync.dma_start(out=ones_sb, in_=ones_h.ap())
                    if it == 0:
                        # warm-up collective: absorbs the cold first-collective
                        # latency so AG(k^T ch0) runs warm
                        nc.gpsimd.dma_start(out=cc_wm_in, in_=bl_h.ap())
                        collective("AllGather", ALU.bypass,
                                   [cc_wm_in], [cc_wm_out])

                    # ---------------- phase A: projections ----------------
                    kt_pre = lp.tile([P, DB, JC, P], f16, name="kt_pre")
                    with (
                        tc.tile_pool(name="pw", bufs=1) as pw,
                        tc.tile_pool(name="pst", bufs=4) as pst,
                        tc.tile_pool(name="psA", bufs=3, space="PSUM") as psA,
                        tc.tile_pool(name="psA2", bufs=2, space="PSUM") as psA2,
                    ):
                        WhT_sb = pw.tile([P, DB, D], f16)
                        WlT_sb = pw.tile([P, DB, D], f16)
                        WgT_sb = pw.tile([P, DB, D], f16)
                        pT_sb = pw.tile([P, DB, NL], f16)
                        rT_sb = pw.tile([P, DB, NL], f16)
                        for db in range(DB):
                            for t_sb, t_h in ((WlT_sb, WlT_h), (rT_sb, rT_h)):
                                nc.sync.dma_start(
                                    out=t_sb[:, db, :],
                                    in_=t_h.ap()[db * P:(db + 1) * P, :])
                        for t_sb, t_h in ((WgT_sb, WgT_h), (pT_sb, pT_h),
                                          (WhT_sb, WhT_h)):
                            for db in range(DB):
                                nc.sync.dma_start(
                                    out=t_sb[:, db, :],
                                    in_=t_h.ap()[db * P:(db + 1) * P, :])

                        # k^T shard chunk-major: chunk ch covers keys
                        # [ch*CW, (ch+1)*CW); AG each chunk as soon as done
                        for ch in range(NCH):
                            for dob in range(DB):
                                ps_t = psA.tile([P, CW], f32, tag="psA")
                                for db in range(DB):
                                    nc.tensor.matmul(
                                        ps_t,
                                        lhsT=WlT_sb[:, db, dob * P:(dob + 1) * P],
                                        rhs=rT_sb[:, db, ch * CW:(ch + 1) * CW],
                                        start=(db == 0), stop=(db == DB - 1))
                                # drain PSUM on alternating engines so the
                                # copy+sem latency of one group never backs
                                # up the ring (DVE is idle in phase A)
                                st = pst.tile([P, CW], f16, tag="st")
                                if dob % 2 == 0:
                                    nc.scalar.activation(
                                        out=st, in_=ps_t, func=AF.Identity,
                                        bias=bl_sb[:, dob:dob + 1], scale=1.0)
                                    nc.scalar.dma_start(
                                        out=cc_kt_in[ch][dob * P:(dob + 1) * P, :],
                                        in_=st)
                                else:
                                    nc.vector.tensor_scalar_add(
                                        st, ps_t, bl_sb[:, dob:dob + 1])
                                    nc.sync.dma_start(
                                        out=cc_kt_in[ch][dob * P:(dob + 1) * P, :],
                                        in_=st)
                            collective("AllGather", ALU.bypass,
                                       [cc_kt_in[ch]], [cc_kt_out[ch]])
                            if ch == 0:
                                # prefetch first K^T block (ch0, core0) on sync
                                for db in range(DB):
                                    nc.sync.dma_start(
                                        out=kt_pre[:, db, :, :].rearrange(
                                            "p a b -> p (a b)"),
                                        in_=cc_kt_out[0][0, db * P:(db + 1) * P, :])

                        # v shard = p^T.T @ Wg^T -> [j_l, dv], + bg via ones-row
                        for jb in range(JBL):
                            ps_t = psA2.tile([P, NL], f32, tag="psAv")
                            for db in range(DB):
                                for dvh in range(2):
                                    nc.tensor.matmul(
                                        ps_t[:, dvh * FH:(dvh + 1) * FH],
                                        lhsT=pT_sb[:, db, jb * P:(jb + 1) * P],
                                        rhs=WgT_sb[:, db, dvh * FH:(dvh + 1) * FH],
                                        start=(db == 0), stop=False)
                            for dvh in range(2):
                                nc.tensor.matmul(
                                    ps_t[:, dvh * FH:(dvh + 1) * FH],
                                    lhsT=ones_sb[:, :],
                                    rhs=bg_sb[:, dvh * FH:(dvh + 1) * FH],
                                    start=False, stop=True)
                            st = pst.tile([P, NL], f16, tag="stv")
                            if jb % 2 == 0:
                                nc.scalar.activation(out=st, in_=ps_t,
                                                     func=AF.Copy)
                                nc.scalar.dma_start(
                                    out=cc_v_in[jb * P:(jb + 1) * P, :], in_=st)
                            else:
                                nc.vector.tensor_copy(out=st, in_=ps_t)
                                nc.sync.dma_start(
                                    out=cc_v_in[jb * P:(jb + 1) * P, :], in_=st)
                        collective("AllGather", ALU.bypass, [cc_v_in], [cc_v_out])

                        # q^T = Wh^T.T @ p^T -> [do, i]; stays in SBUF
                        for dob in range(DB):
                            ps_t = psA2.tile([P, NL], f32, tag="psAv")
                            for db in range(DB):
                                for ih in range(2):
                                    nc.tensor.matmul(
                                        ps_t[:, ih * FH:(ih + 1) * FH],
                                        lhsT=WhT_sb[:, db, dob * P:(dob + 1) * P],
                                        rhs=pT_sb[:, db, ih * FH:(ih + 1) * FH],
                                        start=(db == 0), stop=(db == DB - 1))
                            if dob % 2 == 0:
                                nc.scalar.activation(out=qT[:, dob, :],
                                                     in_=ps_t, func=AF.Copy)
                            else:
                                nc.vector.tensor_copy(out=qT[:, dob, :],
                                                      in_=ps_t)

                    # -------- phase C: scores^T + local stats --------
                    # group order: g = ch*32 + c_idx*JC + jl  (chunk-major)
                    ep_cm = tc.tile_pool(name="ep", bufs=1)
                    ep = ep_cm.__enter__()
                    E = ep.tile([P, NG, NL], f16)
                    with (
                        tc.tile_pool(name="ktp", bufs=3) as ktp,
                        tc.tile_pool(name="psC", bufs=2, space="PSUM") as psC,
                    ):
                        for ch in range(NCH):
                            for c_idx in range(NCORES):
                                if ch == 0 and c_idx == 0:
                                    kt_c = kt_pre
                                else:
                                    kt_c = ktp.tile([P, DB, JC, P], f16, tag="kt")
                                    nc.sync.dma_start(
                                        out=kt_c,
                                        in_=cc_kt_out[ch][c_idx].rearrange(
                                            "(a p) (b c) -> p a b c",
                                            p=P, b=JC))
                                for jl in range(JC):
                                    g = ch * (NG // 2) + c_idx * JC + jl
                                    ps_t = psC.tile([P, NL], f32, tag="sc")
                                    for db in range(DB):
                                        for ih in range(2):
                                            nc.tensor.matmul(
                                                ps_t[:, ih * FH:(ih + 1) * FH],
                                                lhsT=kt_c[:, db, jl, :],
                                                rhs=qT[:, db, ih * FH:(ih + 1) * FH],
                                                start=(db == 0),
                                                stop=(db == DB - 1))
                                    nc.vector.tensor_reduce(
                                        out=stats[:, 0, g:g + 1], in_=ps_t,
                                        op=ALU.max, axis=AX.X, negate=True)
                                    nc.scalar.activation(
                                        out=E[:, g, :], in_=ps_t, func=AF.Exp,
                                        bias=stats[:, 0, g:g + 1], scale=1.0,
                                        accum_out=stats[:, 1, g:g + 1])

                    # prefetch first phase-E V tiles (sync queue)
                    vtp_cm = tc.tile_pool(name="vtp", bufs=8)
                    vtp = vtp_cm.__enter__()

                    def v_src(g):
                        c_idx = (g % (NG // 2)) // JC
                        jlb = (g // (NG // 2)) * JC + (g % JC)
                        return cc_v_out[c_idx, jlb * P:(jlb + 1) * P, :]

                    # g=0's V tile lives outside the ring: loaded once, used
                    # by BOTH i-half passes (no reload latency at the pass-2
                    # boundary)
                    vt0 = lp.tile([P, D], f16, name="vt0")
                    nc.sync.dma_start(out=vt0, in_=v_src(0))
                    vt_pre = [vt0]
                    for g in range(1, 6):
                        vt = vtp.tile([P, D], f16, tag="vt", name=f"vtpre{g}")
                        nc.sync.dma_start(out=vt, in_=v_src(g))
                        vt_pre.append(vt)

                    # residual p prefetch (per i-half) on gpsimd: needed only
                    # at pass ends
                    prp_cm = tc.tile_pool(name="prp", bufs=1)
                    prp = prp_cm.__enter__()

                    # stats AllGather + combine in two halves; the first
                    # half's AG/combine/E-scale hide under phase C's tail
                    NH = NG // 2
                    Mneg = lp.tile([P, NG], f32)
                    Ssum = lp.tile([P, NG], f32)
                    tmp = lp.tile([P, NG], f32)
                    diff = lp.tile([P, NG], f32)
                    alpha = lp.tile([P, NG], f32)
                    rec = lp.tile([P, NG], f32)
                    gath = [lp.tile([P, NCORES, 2, NH], f32, name=f"gath{h}")
                            for h in range(2)]
                    for h in range(2):
                        hs = slice(h * NH, (h + 1) * NH)
                        nc.gpsimd.dma_start(out=cc_st_in[h][:, 0:NH],
                                            in_=stats[:, 0, hs])
                        nc.gpsimd.dma_start(out=cc_st_in[h][:, NH:NG],
                                            in_=stats[:, 1, hs])
                        collective("AllGather", ALU.bypass,
                                   [cc_st_in[h]], [cc_st_out[h]])
                        nc.gpsimd.dma_start(
                            out=gath[h].rearrange("p c a b -> p (c a b)"),
                            in_=cc_st_out[h].rearrange("c p x -> p c x"))
                        g_h = gath[h]
                        nc.vector.tensor_copy(out=Mneg[:, hs],
                                              in_=g_h[:, 0, 0, :])
                        for c in range(1, NCORES):
                            nc.vector.tensor_tensor(out=Mneg[:, hs],
                                                    in0=Mneg[:, hs],
                                                    in1=g_h[:, c, 0, :],
                                                    op=ALU.min)
                        for c in range(NCORES):
                            nc.vector.tensor_sub(out=tmp[:, hs],
                                                 in0=Mneg[:, hs],
                                                 in1=g_h[:, c, 0, :])
                            nc.scalar.activation(out=tmp[:, hs],
                                                 in_=tmp[:, hs], func=AF.Exp)
                            nc.vector.tensor_mul(out=tmp[:, hs],
                                                 in0=tmp[:, hs],
                                                 in1=g_h[:, c, 1, :])
                            if c == 0:
                                nc.vector.tensor_copy(out=Ssum[:, hs],
                                                      in_=tmp[:, hs])
                            else:
                                nc.vector.tensor_add(out=Ssum[:, hs],
                                                     in0=Ssum[:, hs],
                                                     in1=tmp[:, hs])
                        # f = exp(Mneg - mneg_local) / Ssum, fold into E (DVE)
                        nc.vector.tensor_sub(out=diff[:, hs], in0=Mneg[:, hs],
                                             in1=stats[:, 0, hs])
                        nc.scalar.activation(out=alpha[:, hs], in_=diff[:, hs],
                                             func=AF.Exp)
                        nc.vector.reciprocal(out=rec[:, hs], in_=Ssum[:, hs])
                        nc.vector.tensor_mul(out=f_sc[:, hs], in0=alpha[:, hs],
                                             in1=rec[:, hs])
                        for g in range(h * NH, (h + 1) * NH):
                            nc.vector.tensor_scalar_mul(
                                E[:, g, :], E[:, g, :], f_sc[:, g:g + 1])

                    # -------- phase E: out = E^T.T @ V + p (two i-half passes)
                    with (
                        tc.tile_pool(name="osp", bufs=3) as osp,
                        tc.tile_pool(name="psE", bufs=1, space="PSUM") as psE,
                    ):
                        for ihalf in range(2):
                            po = [psE.tile([P, D], f32, tag=f"po{q_}",
                                           name=f"po{q_}")
                                  for q_ in range(IB // 2)]
                            pr_sb = prp.tile([P, IB // 2, D], f32, tag="pr")
                            for q_ in range(IB // 2):
                                ib = ihalf * (IB // 2) + q_
                                nc.gpsimd.dma_start(
                                    out=pr_sb[:, q_, :],
                                    in_=pres_h.ap()[ib * P:(ib + 1) * P, :])
                            for g in range(NG):
                                if g == 0:
                                    vt = vt0
                                elif ihalf == 0 and g < 6:
                                    vt = vt_pre[g]
                                else:
                                    vt = vtp.tile([P, D], f16, tag="vt")
                                    nc.sync.dma_start(out=vt, in_=v_src(g))
                                for q_ in range(IB // 2):
                                    ib = ihalf * (IB // 2) + q_
                                    for dvh in range(2):
                                        nc.tensor.matmul(
                                            po[q_][:, dvh * FH:(dvh + 1) * FH],
                                            lhsT=E[:, g, ib * P:(ib + 1) * P],
                                            rhs=vt[:, dvh * FH:(dvh + 1) * FH],
                                            start=(g == 0), stop=(g == NG - 1))
                            for q_ in range(IB // 2):
                                ib = ihalf * (IB // 2) + q_
                                ot = osp.tile([P, D], f32, tag="ot")
                                nc.vector.tensor_add(out=ot, in0=po[q_],
                                                     in1=pr_sb[:, q_, :])
                                eng = nc.scalar if q_ % 2 == 0 else nc.sync
                                eng.dma_start(
                                    out=out_h.ap()[ib * P:(ib + 1) * P, :],
                                    in_=ot)
                    prp_cm.__exit__(None, None, None)
                    vtp_cm.__exit__(None, None, None)
                    ep_cm.__exit__(None, None, None)
            if spin_us:
                with tc.tile_critical():
                    for _ in range(spin_us):
                        nc.vector.nop(cycle_cnt=960)
    nc.compile()
    return nc


def prepare_in_maps(p, r, Wh, bh, Wl, bl, Wg, bg):
    f16 = np.float16
    f32 = np.float32
    WhT = np.ascontiguousarray(Wh.T).astype(f16)
    WlT = np.ascontiguousarray(Wl.T).astype(f16)
    WgT = np.ascontiguousarray(Wg.T).astype(f16)
    bl_r = np.ascontiguousarray(bl.astype(f32).reshape(DB, P).T)
    bg16 = bg.astype(f16).reshape(1, D)
    in_maps = []
    for c in range(NCORES):
        sl = slice(c * NL, (c + 1) * NL)
        in_maps.append({
            "pT": np.ascontiguousarray(p[sl].T).astype(f16),
            "rT": np.ascontiguousarray(r[sl].T).astype(f16),
            "pres": np.ascontiguousarray(p[sl]).astype(f32),
            "WhT": WhT, "WlT": WlT, "WgT": WgT,
            "bl_r": bl_r, "bg16": bg16, "ones16": np.ones((1, P), f16),
        })
    return in_maps


_NC_CACHE = {}


def kernel(p, r, Wh, bh, Wl, bl, Wg, bg):
    from concourse.bass_utils import run_bass_kernel_spmd

    p = np.asarray(p); r = np.asarray(r)
    in_maps = prepare_in_maps(p, r, np.asarray(Wh), np.asarray(bh),
                              np.asarray(Wl), np.asarray(bl),
                              np.asarray(Wg), np.asarray(bg))
    if 1 not in _NC_CACHE:
        _NC_CACHE[1] = build_nc(1)
    res = run_bass_kernel_spmd(_NC_CACHE[1], in_maps, list(range(NCORES)))
    out = np.concatenate([res.results[c]["out"] for c in range(NCORES)], axis=0)
    return out.astype(np.float32)
